# revision 16
# baseline (speedup 1.0000x reference)
"""GCNConv (N=20000, E=320000, D=1024) on 8 trn2 NeuronCores.

out = segment_sum(norm * h[col] -> row),  h = x @ W^T + b,
norm = deg^-1/2[row] * deg^-1/2[col], with self-loops added.

Sharding: nodes split 2500/core (padded to 2560 = 20 blocks of 128).
Per core: h_scaled = (dis*x) @ W^T + dis (x) b  (bf16 matmul + exact fp32
rank-1 bias, bf16 store).  The AllGather of h_scaled runs in CH chunks
(one Shared table per chunk) interleaved with the h compute; edges are
partitioned by destination block and sub-partitioned by source chunk, so
gathers (bulk dma_gather) for chunk 0 start while later chunks are still
all-gathering.  Segment-sum per (block, chunk): 0/1 selection matrices
via iota==dest_local, PE matmul accumulate in fp32 PSUM, accumulated
across chunks in an fp32 SBUF accumulator, scaled by dis[dest] on the
way out.
"""

import numpy as np
import ml_dtypes

import concourse.bacc as bacc
import concourse.mybir as mybir
import concourse.tile as tile
from concourse import bass
from concourse import bass_utils

N = 20000
E = 320000
D = 1024
NC = 8
NPC = N // NC            # 2500 real nodes per core
NBLK = 20                # dest blocks of 128 per core
NPCP = NBLK * 128        # 2560 padded nodes per core
P = 128
KT = D // P              # 8 contraction tiles
CH = 2                   # AllGather chunks

_cache = {}


def _preprocess(x, edge_index, W, b):
    x = np.asarray(x, dtype=np.float32)
    ei = np.asarray(edge_index)
    W = np.asarray(W, dtype=np.float32)
    b = np.asarray(b, dtype=np.float32)

    self_idx = np.arange(N, dtype=np.int64)
    rowD = np.concatenate([ei[0].astype(np.int64), self_idx])
    deg = np.bincount(rowD, minlength=N).astype(np.float32)
    dis = deg ** -0.5  # deg >= 1 (self loops)
    # appended self-loops are folded into the h phase (out += dis*h_scaled),
    # so the gather segments hold only the original edges
    row = ei[0].astype(np.int64)
    col = ei[1].astype(np.int64)

    CHe = min(CH, NBLK)
    RCH = NPCP // CHe
    csrc = (col // NPC).astype(np.int64)
    l = (col % NPC).astype(np.int64)
    chk = (l // RCH).astype(np.int32)          # source chunk of each edge
    wcr = (csrc * RCH + (l - chk * RCH)).astype(np.int32)  # row in chunk table

    core_of = (row // NPC).astype(np.int32)
    rl = (row - core_of.astype(np.int64) * NPC).astype(np.int32)  # local dest

    # per (core, block, chunk) edge lists
    seg_cols = {}
    seg_dl = {}
    for c in range(NC):
        m = core_of == c
        rl_c = rl[m]
        wcr_c = wcr[m]
        chk_c = chk[m]
        blk = rl_c // P
        key = blk.astype(np.int64) * CHe + chk_c
        order = np.argsort(key, kind="stable")
        rl_c, wcr_c, key = rl_c[order], wcr_c[order], key[order]
        bounds = np.searchsorted(key, np.arange(NBLK * CHe + 1))
        for bk in range(NBLK):
            for ch in range(CHe):
                s, e = bounds[bk * CHe + ch], bounds[bk * CHe + ch + 1]
                o2 = np.argsort(wcr_c[s:e], kind="stable")
                seg_cols[(c, bk, ch)] = wcr_c[s:e][o2]
                seg_dl[(c, bk, ch)] = (rl_c[s:e] - bk * P)[o2]

    # shared schedule: T[(b,ch)] = max over cores of ceil(edges/128)
    T_mat = []
    for bk in range(NBLK):
        rowT = []
        for ch in range(CHe):
            mx = max(len(seg_cols[(c, bk, ch)]) for c in range(NC))
            rowT.append(-(-mx // P))
        T_mat.append(tuple(rowT))
    T_mat = tuple(T_mat)
    NT = sum(sum(r) for r in T_mat)

    # canonical tile order: b-major, then chunk, then tiles
    dl = np.full((NC, NT, P), -1.0, dtype=np.float32)
    idx16 = np.zeros((NC, 16, NT * 8), dtype=np.int16)
    t0 = 0
    for bk in range(NBLK):
        for ch in range(CHe):
            Tb = T_mat[bk][ch]
            if Tb == 0:
                continue
            for c in range(NC):
                cc = seg_cols[(c, bk, ch)]
                dd = seg_dl[(c, bk, ch)]
                n = len(cc)
                flat_d = dl[c, t0 : t0 + Tb].reshape(-1)
                flat_d[:n] = dd.astype(np.float32)
                buf = np.zeros(Tb * P, dtype=np.int16)
                buf[:n] = cc.astype(np.int16)
                idx16[c, :, t0 * 8 : (t0 + Tb) * 8] = buf.reshape(Tb * 8, 16).T
            t0 += Tb

    WTb = np.zeros((D + P, D), dtype=ml_dtypes.bfloat16)  # [d(+bias), o]
    WTb[:D] = W.T.astype(ml_dtypes.bfloat16)
    WTb[D:] = (b / P).astype(ml_dtypes.bfloat16)[None, :]
    bvec = b.reshape(1, D)

    in_maps = []
    for c in range(NC):
        rows = slice(c * NPC, (c + 1) * NPC)
        dis_c = dis[rows]
        xs = x[rows] * dis_c[:, None]
        xT = np.zeros((D + P, NPCP), dtype=ml_dtypes.bfloat16)
        xT[:D, :NPC] = xs.T.astype(ml_dtypes.bfloat16)
        xT[D:, :NPC] = dis_c.astype(ml_dtypes.bfloat16)[None, :]
        disv = np.zeros((1, NPCP), dtype=np.float32)
        disv[0, :NPC] = dis_c
        disT = np.ascontiguousarray(disv.reshape(NBLK, P).T)  # [128, NBLK]
        in_maps.append(
            {
                "xT": xT,
                "WT": WTb,
                "bvec": bvec,
                "disv": disv,
                "disT": disT,
                "idx16": np.ascontiguousarray(np.tile(idx16[c], (8, 1))),
                "dl": np.ascontiguousarray(dl[c].T),  # [128, NT]
            }
        )
    return T_mat, NT, in_maps


def _build(T_mat, NT):
    f32 = mybir.dt.float32
    bf16 = mybir.dt.bfloat16
    i16 = mybir.dt.int16
    i32 = mybir.dt.int32
    CHe = min(CH, NBLK)
    RCH = NPCP // CHe
    JCH = NBLK // CHe  # h blocks per AG chunk

    # canonical tile offsets (b-major)
    tstart = {}
    t0 = 0
    for bk in range(NBLK):
        for ch in range(CHe):
            tstart[(bk, ch)] = t0
            t0 += T_mat[bk][ch]
    last_ch = {}
    for bk in range(NBLK):
        nz = [ch for ch in range(CHe) if T_mat[bk][ch] > 0]
        last_ch[bk] = nz[-1] if nz else -1

    nc = bacc.Bacc("TRN2", target_bir_lowering=False, debug=False, num_devices=NC, num_swdge_queues=2)
    xT = nc.dram_tensor("xT", [D + P, NPCP], bf16, kind="ExternalInput").ap()
    WT = nc.dram_tensor("WT", [D + P, D], bf16, kind="ExternalInput").ap()
    bvec = nc.dram_tensor("bvec", [1, D], f32, kind="ExternalInput").ap()
    disv = nc.dram_tensor("disv", [1, NPCP], f32, kind="ExternalInput").ap()
    disT = nc.dram_tensor("disT", [P, NBLK], f32, kind="ExternalInput").ap()
    idx16 = nc.dram_tensor("idx16", [P, NT * 8], i16, kind="ExternalInput").ap()
    dl = nc.dram_tensor("dl", [P, NT], f32, kind="ExternalInput").ap()
    yout = nc.dram_tensor("yout", [NPCP, D], f32, kind="ExternalOutput").ap()

    with tile.TileContext(nc) as tc:
        with tc.tile_pool(name="dram", bufs=1, space="DRAM") as dram, \
             tc.tile_pool(name="const", bufs=1) as const:
            h_ch = [dram.tile([RCH, D], bf16, name=f"h_ch{c_}") for c_ in range(CHe)]
            hg_ch = [
                dram.tile([NC * RCH, D], bf16, addr_space="Shared", name=f"hg_ch{c_}")
                for c_ in range(CHe)
            ]

            wt_sb = const.tile([P, (KT + 1) * D], bf16, name="wt_sb")
            for k in range(KT + 1):
                nc.sync.dma_start(
                    wt_sb[:, k * D : (k + 1) * D], WT[k * P : (k + 1) * P, :]
                )
            disT_sb = const.tile([P, NBLK], f32, name="disT_sb")
            nc.sync.dma_start(disT_sb[:], disT[:])
            ix_sb = const.tile([P, NT * 8], i16, name="ix_sb")
            nc.sync.dma_start(ix_sb[:], idx16[:])
            dl_sb = const.tile([P, NT], f32, name="dl_sb")
            nc.sync.dma_start(dl_sb[:], dl[:])
            TMAX = max(max(r) for r in T_mat)
            iota_rep = const.tile([P, TMAX * P], f32, name="iota_rep")
            with tc.tile_pool(name="tmpi", bufs=1) as tmpp:
                iota_i = tmpp.tile([P, TMAX * P], i32, name="iota_i")
                nc.gpsimd.iota(
                    iota_i[:], pattern=[[0, TMAX], [1, P]], channel_multiplier=0
                )
                nc.vector.tensor_copy(iota_rep[:], iota_i[:])

            acc_cm = tc.tile_pool(name="acc", bufs=1)
            accp = acc_cm.__enter__()
            acc = accp.tile([P, NBLK, D], f32, name="acc")

            # ---------------- h phase (+ chunked AllGather) ----------------
            with tc.tile_pool(name="xk", bufs=1) as xkp, \
                 tc.tile_pool(name="hps", bufs=2, space="PSUM") as hps, \
                 tc.tile_pool(name="hout", bufs=3) as houtp:
                xk_sb = xkp.tile([P, (KT + 1) * NPCP], bf16, name="xk_sb")
                for k in range(KT + 1):
                    nc.sync.dma_start(
                        xk_sb[:, k * NPCP : (k + 1) * NPCP],
                        xT[k * P : (k + 1) * P, :],
                    )
                chunks = [slice(s, min(s + 512, D)) for s in range(0, D, 512)]
                for j in range(NBLK):
                    ps = hps.tile([P, D], f32)
                    for k in range(KT + 1):
                        lhsT = xk_sb[:, k * NPCP + j * P : k * NPCP + (j + 1) * P]
                        for cs in chunks:
                            nc.tensor.matmul(
                                ps[:, cs],
                                lhsT=lhsT,
                                rhs=wt_sb[:, k * D + cs.start : k * D + cs.stop],
                                start=(k == 0),
                                stop=(k == KT),
                            )
                    hsb = houtp.tile([P, D], bf16)
                    nc.vector.tensor_copy(hsb[:], ps[:])
                    nc.vector.tensor_copy(acc[:, j, :], ps[:])
                    ch = j // JCH
                    jo = j - ch * JCH
                    nc.sync.dma_start(h_ch[ch][jo * P : (jo + 1) * P, :], hsb[:])
                    if jo == JCH - 1:
                        nc.gpsimd.collective_compute(
                            "AllGather",
                            mybir.AluOpType.bypass,
                            replica_groups=[list(range(NC))],
                            ins=[h_ch[ch][:]],
                            outs=[hg_ch[ch][:]],
                        )

            # ---------------- aggregation phase ----------------
            with tc.tile_pool(name="gath", bufs=4) as gp, \
                 tc.tile_pool(name="sel", bufs=3) as selp, \
                 tc.tile_pool(name="aps", bufs=3, space="PSUM") as aps, \
                 tc.tile_pool(name="aout", bufs=3) as aoutp:
                GSUB = 8
                gq = 0  # <=1024 idxs per single-packet dma_gather
                for ch in range(CHe):
                    for bk in range(NBLK):
                        Tb = T_mat[bk][ch]
                        if Tb == 0:
                            continue
                        t0 = tstart[(bk, ch)]
                        g = gp.tile([P, Tb, D], bf16, tag="g")
                        for s0 in range(0, Tb, GSUB):
                            sn = min(GSUB, Tb - s0)
                            nc.gpsimd.dma_gather(
                                g[:, s0 : s0 + sn, :],
                                hg_ch[ch][:],
                                ix_sb[:, (t0 + s0) * 8 : (t0 + s0 + sn) * 8],
                                sn * P,
                                sn * P,
                                D,
                                queue_num=gq,
                            )
                            gq = 1 - gq
                        selb = selp.tile([P, Tb, P], bf16, tag="selb")
                        dlb = (
                            dl_sb[:, t0 : t0 + Tb]
                            .rearrange("p (t o) -> p t o", o=1)
                            .to_broadcast([P, Tb, P])
                        )
                        nc.vector.tensor_tensor(
                            out=selb[:],
                            in0=iota_rep[:, : Tb * P].rearrange(
                                "p (t o) -> p t o", o=P
                            ),
                            in1=dlb,
                            op=mybir.AluOpType.is_equal,
                        )
                        ps = aps.tile([P, D], f32)
                        for i in range(Tb):
                            for cs in [
                                slice(s, min(s + 512, D)) for s in range(0, D, 512)
                            ]:
                                nc.tensor.matmul(
                                    ps[:, cs],
                                    lhsT=selb[:, i, :],
                                    rhs=g[:, i, cs],
                                    start=(i == 0),
                                    stop=(i == Tb - 1),
                                )
                        nc.vector.tensor_add(
                            out=acc[:, bk, :], in0=acc[:, bk, :], in1=ps[:]
                        )
                        if ch == last_ch[bk]:
                            ob = aoutp.tile([P, D], f32)
                            nc.vector.tensor_scalar(
                                out=ob[:],
                                in0=acc[:, bk, :],
                                scalar1=disT_sb[:, bk : bk + 1],
                                scalar2=None,
                                op0=mybir.AluOpType.mult,
                            )
                            nc.sync.dma_start(yout[bk * P : (bk + 1) * P, :], ob[:])
                for bk in range(NBLK):
                    if last_ch[bk] == -1:
                        ob = aoutp.tile([P, D], f32, tag="ob")
                        nc.vector.tensor_scalar(
                            out=ob[:],
                            in0=acc[:, bk, :],
                            scalar1=disT_sb[:, bk : bk + 1],
                            scalar2=None,
                            op0=mybir.AluOpType.mult,
                        )
                        nc.sync.dma_start(yout[bk * P : (bk + 1) * P, :], ob[:])
            acc_cm.__exit__(None, None, None)

    nc.compile()
    return nc


def kernel(x, edge_index, W, b):
    T_mat, NT, in_maps = _preprocess(x, edge_index, W, b)
    key = (T_mat, NT)
    if key not in _cache:
        _cache[key] = _build(T_mat, NT)
    nc = _cache[key]
    res = bass_utils.run_bass_kernel_spmd(nc, in_maps, core_ids=list(range(NC)))
    out = np.empty((N, D), dtype=np.float32)
    for c in range(NC):
        out[c * NPC : (c + 1) * NPC] = res.results[c]["yout"][:NPC]
    return out
